# revision 1
# baseline (speedup 1.0000x reference)
"""Trainium2 Bass kernel for nn_CustomGNN (GCN + GMT pooling, 3 layers).

Sharding: data-parallel over graphs (16 graphs / core, 8 cores).
- Dense-aligned node layout per core: 16 graphs x 384 slots = 6144 rows.
- GCN aggregation: edges (incl. self-loops, weight 1/deg) sharded by dst
  core, sorted by dense dst slot, processed as 128-edge chunks:
  indirect-gather of xw rows (bf16, 4 rows/partition per DMA) + selection
  matrix M built on DVE (iota/is_equal*norm) + TensorE matmul accumulation.
- xw exchanged between cores once per layer via AllGather (bf16).
- Attention (PMA with host-folded query, SAB) in fp32, feat-major xT
  maintained across layers via combT matmuls (no full re-transpose).
"""
import os
import numpy as np
from contextlib import ExitStack

import concourse.bass as bass
import concourse.tile as tile
from concourse import bacc, mybir
from concourse.bass_utils import run_bass_kernel_spmd
from concourse.masks import make_identity

P = 128
NCORES = 8
NUM_GRAPHS = 128
GPC = NUM_GRAPHS // NCORES      # 16 graphs per core
MAX_N = 384
NT = MAX_N // P                 # 3 node tiles per graph
DSLOT = GPC * MAX_N             # 6144 dense slots per core
NBLK = DSLOT // P               # 48 dst blocks per core
N_NODES = 32768
D = 256
KC = D // P                     # 2 feature chunks
C_SEEDS = 32
H_HEADS = 4
L_LAYERS = 3
D_OUT = 32
GROW = 4                        # rows gathered per partition per indirect DMA
ISCALE = 1.0 / 16.0             # 1/sqrt(D)

f32 = mybir.dt.float32
bf16 = mybir.dt.bfloat16
i32 = mybir.dt.int32
AF = mybir.ActivationFunctionType
ALU = mybir.AluOpType
AX = mybir.AxisListType

LAST_EXEC_TIME_NS = None
_CACHE = {}


# ----------------------------------------------------------------------------
# Host preprocessing (index/structure only + weights-only folding)
# ----------------------------------------------------------------------------

def _preprocess(x, edge_index, batch_ids, seeds, proj_W, proj_b):
    src = np.asarray(edge_index[0]).astype(np.int64)
    dst = np.asarray(edge_index[1]).astype(np.int64)
    batch_ids = np.asarray(batch_ids).astype(np.int64)
    counts = np.bincount(batch_ids, minlength=NUM_GRAPHS)
    starts = np.cumsum(counts) - counts
    pos = np.arange(N_NODES, dtype=np.int64) - starts[batch_ids]
    gcore = batch_ids // GPC
    glocal = batch_ids % GPC
    dslot = glocal * MAX_N + pos                      # [N] slot within core
    gslot = gcore * DSLOT + dslot                     # [N] global dense slot

    deg = 1.0 + np.bincount(dst, minlength=N_NODES).astype(np.float64)
    dis = 1.0 / np.sqrt(deg)
    # append self loops with weight 1/deg (the xw/deg self term)
    src_a = np.concatenate([src, np.arange(N_NODES, dtype=np.int64)])
    dst_a = np.concatenate([dst, np.arange(N_NODES, dtype=np.int64)])
    w_a = np.concatenate([(dis[src] * dis[dst]).astype(np.float32),
                          (1.0 / deg).astype(np.float32)])

    ecore = gcore[dst_a]
    per_core = []
    blk_counts = np.zeros((NCORES, NBLK), np.int64)
    for c in range(NCORES):
        m = ecore == c
        es, ed, ew = src_a[m], dst_a[m], w_a[m]
        sd = dslot[ed]
        order = np.argsort(sd, kind="stable")
        sd = sd[order]
        ssrc = gslot[es[order]].astype(np.int32)
        ew = ew[order]
        blk_counts[c] = np.bincount(sd // P, minlength=NBLK)
        per_core.append((ssrc, (sd % P).astype(np.float32), ew))

    nch = np.maximum(1, np.ceil(blk_counts / P).astype(np.int64).max(axis=0))
    TCH = int(nch.sum())
    TCH_PAD = ((TCH + GROW - 1) // GROW) * GROW
    nch[NBLK - 1] += TCH_PAD - TCH
    chbase = np.cumsum(nch) - nch

    esrc = np.zeros((NCORES, P, TCH_PAD), np.int32)
    eldst = np.zeros((NCORES, P, TCH_PAD), np.float32)
    ew_arr = np.zeros((NCORES, P, TCH_PAD), np.float32)
    for c in range(NCORES):
        ssrc, ld, ew = per_core[c]
        blk = np.repeat(np.arange(NBLK), blk_counts[c])
        bstart = np.cumsum(blk_counts[c]) - blk_counts[c]
        k = np.arange(len(ssrc)) - bstart[blk]
        col = chbase[blk] + k // P
        row = k % P
        esrc[c, row, col] = ssrc
        eldst[c, row, col] = ld
        ew_arr[c, row, col] = ew

    negmask = np.zeros((NCORES, 1, DSLOT), np.float32)
    cnts = counts.reshape(NCORES, GPC)
    sl = np.arange(DSLOT)
    for c in range(NCORES):
        real = sl % MAX_N < cnts[c][sl // MAX_N]
        negmask[c, 0, ~real] = -1e9

    xT = np.zeros((NCORES, D, DSLOT), np.float32)
    xx = np.asarray(x)
    for c in range(NCORES):
        idx = np.where(gcore == c)[0]
        xT[c][:, dslot[idx]] = xx[idx].T

    # host-folded PMA query (weights-only)
    qbd = np.zeros((L_LAYERS, D, P), np.float32)
    qcat = np.zeros((L_LAYERS, C_SEEDS, D), np.float32)
    dh = D // H_HEADS
    for l in range(L_LAYERS):
        qc = np.asarray(seeds[l]) @ np.asarray(proj_W[l][0]) + np.asarray(proj_b[l][0])
        qcat[l] = qc
        for h in range(H_HEADS):
            qbd[l, h * dh:(h + 1) * dh, h * C_SEEDS:(h + 1) * C_SEEDS] = \
                qc[:, h * dh:(h + 1) * dh].T * ISCALE

    # comb exchange gather indices: graph b2 needs A-slices at flat
    # indices 4*b2+j (flat = h*NUM_GRAPHS + b), stored in ag2 layout
    # rowbase = (b//GPC)*2048 + (h*GPC + b%GPC)*32.
    es2 = np.zeros((NCORES, C_SEEDS, 4 * GPC), np.int32)
    for c in range(NCORES):
        for g in range(GPC):
            b2 = GPC * c + g
            for j in range(4):
                mflat = 4 * b2 + j
                h, b = mflat // NUM_GRAPHS, mflat % NUM_GRAPHS
                rowbase = (b // GPC) * 2048 + (h * GPC + (b % GPC)) * 32
                es2[c, :, 4 * g + j] = rowbase + np.arange(C_SEEDS)

    meta = dict(nch=nch, gcore=gcore, dslot=dslot)
    return meta, dict(esrc=esrc, eldst=eldst, ew=ew_arr, negmask=negmask,
                      xT=xT, qbd=qbd, qcat=qcat, es2=es2)


# ----------------------------------------------------------------------------
# Device kernel
# ----------------------------------------------------------------------------

def _build(TCH, nch):
    nc = bacc.Bacc("TRN2", target_bir_lowering=False, debug=False,
                   num_devices=NCORES)

    def din(name, shape, dt=f32):
        return nc.dram_tensor(name, shape, dt, kind="ExternalInput")

    xT0_d = din("xT0", [D, DSLOT])
    esrc_d = din("esrc", [P, TCH], i32)
    eldst_d = din("eldst", [P, TCH])
    ew_d = din("ew", [P, TCH])
    negmask_d = din("negmask", [1, DSLOT])
    es2_d = din("es2", [C_SEEDS, 4 * GPC], i32)
    qbd_d = din("qbd", [L_LAYERS, D, P])
    qcat_d = din("qcat", [L_LAYERS, C_SEEDS, D])
    gcnW_d = din("gcnW", [L_LAYERS, D, D])
    gcnb_d = din("gcnb", [L_LAYERS, 1, D])
    pW1_d = din("pW1", [L_LAYERS, D, D])
    pb1T_d = din("pb1T", [L_LAYERS, P, KC])
    pW2_d = din("pW2", [L_LAYERS, D, D])
    pb2_d = din("pb2", [L_LAYERS, 1, D])
    pW3_d = din("pW3", [L_LAYERS, D, D])
    pb3_d = din("pb3", [L_LAYERS, 1, D])
    pln_d = din("pln", [L_LAYERS, 2, 2, P, D])
    eW_d = din("eW", [L_LAYERS, 4, D, D])
    eb0_d = din("eb0", [L_LAYERS, 1, D])
    eb0T_d = din("eb0T", [L_LAYERS, P, KC])
    eb1T_d = din("eb1T", [L_LAYERS, P, KC])
    eb2_d = din("eb2", [L_LAYERS, 1, D])
    eb3_d = din("eb3", [L_LAYERS, 1, D])
    eln_d = din("eln", [L_LAYERS, 2, 2, P, D])
    headW_d = din("headW", [D, D_OUT])
    headb_d = din("headb", [P, D_OUT])

    y_d = nc.dram_tensor("y", [DSLOT, D_OUT], f32, kind="ExternalOutput")
    dbg = os.environ.get("ANT_DEBUG", "0") == "1"
    if dbg:
        dbg_xg = nc.dram_tensor("dbg_xg", [DSLOT, D], f32,
                                kind="ExternalOutput")
        dbg_xt = nc.dram_tensor("dbg_xt", [D, DSLOT], f32,
                                kind="ExternalOutput")
        dbg_vns = nc.dram_tensor("dbg_vns", [GPC * C_SEEDS, D], f32,
                                 kind="ExternalOutput")
        dbg_va = nc.dram_tensor("dbg_va", [GPC * C_SEEDS, D], f32,
                                kind="ExternalOutput")
        dbg_A = nc.dram_tensor("dbg_A", [GPC * P, MAX_N], f32,
                               kind="ExternalOutput")

    chbase = np.cumsum(nch) - nch
    dh = D // H_HEADS

    with tile.TileContext(nc) as tc, ExitStack() as ctx:
        cst = ctx.enter_context(tc.tile_pool(name="cst", bufs=1))
        wp = ctx.enter_context(tc.tile_pool(name="wp", bufs=1))
        xTp = ctx.enter_context(tc.tile_pool(name="xTp", bufs=1))
        xg = ctx.enter_context(tc.tile_pool(name="xg", bufs=NBLK))
        xwb = ctx.enter_context(tc.tile_pool(name="xwb", bufs=3))
        gat = ctx.enter_context(tc.tile_pool(name="gat", bufs=12))
        msel = ctx.enter_context(tc.tile_pool(name="msel", bufs=6))
        att = ctx.enter_context(tc.tile_pool(name="att", bufs=2))
        smp = ctx.enter_context(tc.tile_pool(name="smp", bufs=2))
        vnsp = ctx.enter_context(tc.tile_pool(name="vnsp", bufs=GPC))
        dram = ctx.enter_context(tc.tile_pool(name="dram", bufs=1, space="DRAM"))
        psA = ctx.enter_context(tc.tile_pool(name="psA", bufs=2, space="PSUM"))
        psB = ctx.enter_context(tc.tile_pool(name="psB", bufs=2, space="PSUM"))
        psC = ctx.enter_context(tc.tile_pool(name="psC", bufs=2, space="PSUM"))
        psT = ctx.enter_context(tc.tile_pool(name="psT", bufs=2, space="PSUM"))

        ag_in = dram.tile([DSLOT, D], bf16, tag="agin")
        ag2_in = dram.tile([4 * GPC * C_SEEDS, MAX_N], bf16, tag="ag2in")

        # ---- constants ----
        ident = cst.tile([P, P], f32, tag="ident")
        make_identity(nc, ident[:])
        iota_i = cst.tile([P, P], i32, tag="ioti")
        nc.gpsimd.iota(iota_i[:], pattern=[[1, P]], base=0, channel_multiplier=0)
        iota_f = cst.tile([P, P], f32, tag="iotf")
        nc.vector.tensor_copy(iota_f[:], iota_i[:])
        ones_r = cst.tile([1, P], f32, tag="ones")
        nc.vector.memset(ones_r[:], 1.0)
        eps_t = cst.tile([P, 1], f32, tag="eps")
        nc.vector.memset(eps_t[:], 1e-5)

        esrc_t = cst.tile([P, TCH], i32, tag="esrc")
        nc.sync.dma_start(esrc_t[:], esrc_d[:])
        eldst_t = cst.tile([P, TCH], f32, tag="eldst")
        nc.sync.dma_start(eldst_t[:], eldst_d[:])
        ew_t = cst.tile([P, TCH], f32, tag="ew")
        nc.sync.dma_start(ew_t[:], ew_d[:])
        es2_t = cst.tile([C_SEEDS, 4 * GPC], i32, tag="es2")
        nc.sync.dma_start(es2_t[:], es2_d[:])
        headW_t = cst.tile([P, KC * D_OUT], f32, tag="headW")
        for kc in range(KC):
            nc.sync.dma_start(headW_t[:, kc * D_OUT:(kc + 1) * D_OUT],
                              headW_d[kc * P:(kc + 1) * P, :])
        headb_t = cst.tile([P, D_OUT], f32, tag="headb")
        nc.sync.dma_start(headb_t[:], headb_d[:])

        # persistent xT tiles [16][2] of [128, 384]
        xT = [[xTp.tile([P, MAX_N], f32, tag=f"xT_{g}_{kc}",
                        name=f"xT_{g}_{kc}")
               for kc in range(KC)] for g in range(GPC)]
        for g in range(GPC):
            for kc in range(KC):
                nc.sync.dma_start(
                    xT[g][kc][:],
                    xT0_d[kc * P:(kc + 1) * P, g * MAX_N:(g + 1) * MAX_N])

        C = C_SEEDS

        def ln_norm(x_t, g_bc, b_bc, out_t):
            s1 = att.tile([C, 1], f32, tag="ln_s1")
            nc.vector.tensor_reduce(s1[:], x_t[:], axis=AX.X, op=ALU.add)
            m = att.tile([C, 1], f32, tag="ln_m")
            nc.scalar.mul(m[:], s1[:], 1.0 / D)
            xm = att.tile([C, D], f32, tag="ln_xm")
            nc.vector.tensor_scalar(out=xm[:], in0=x_t[:], scalar1=m[:, 0:1],
                                    scalar2=None, op0=ALU.subtract)
            sq = att.tile([C, D], f32, tag="ln_sq", bufs=1)
            vs = att.tile([C, 1], f32, tag="ln_vs")
            nc.scalar.activation(sq[:], xm[:], AF.Square, accum_out=vs[:, 0:1])
            sd = att.tile([C, 1], f32, tag="ln_sd")
            nc.scalar.activation(sd[:], vs[:], AF.Sqrt, bias=eps_t[0:C, 0:1],
                                 scale=1.0 / D)
            rsd = att.tile([C, 1], f32, tag="ln_rsd")
            nc.vector.reciprocal(rsd[:], sd[:])
            nc.vector.tensor_scalar(out=xm[:], in0=xm[:], scalar1=rsd[:, 0:1],
                                    scalar2=None, op0=ALU.mult)
            nc.vector.tensor_tensor(out=xm[:], in0=xm[:], in1=g_bc[0:C, :],
                                    op=ALU.mult)
            nc.vector.tensor_tensor(out=out_t[:], in0=xm[:], in1=b_bc[0:C, :],
                                    op=ALU.add)

        def ffn_block(in_t, W_t, b_row, out_t):
            tT = []
            for kc in range(KC):
                tp = psT.tile([P, C], f32, tag="tr")
                nc.tensor.transpose(tp[:], in_t[:, kc * P:(kc + 1) * P], ident[0:C, 0:C])
                ts_ = att.tile([P, C], f32, tag="ffn_tT")
                nc.scalar.copy(ts_[:], tp[:])
                tT.append(ts_)
            ps = psC.tile([C, D], f32, tag="psC")
            for kc in range(KC):
                nc.tensor.matmul(ps[:], lhsT=tT[kc][:],
                                 rhs=W_t[:, kc * D:(kc + 1) * D],
                                 start=(kc == 0), stop=False)
            nc.tensor.matmul(ps[:], lhsT=ones_r[:, 0:C], rhs=b_row[:],
                             start=False, stop=True)
            r = att.tile([C, D], f32, tag="ffn_r", bufs=1)
            nc.scalar.activation(r[:], ps[:], AF.Relu)
            nc.vector.tensor_tensor(out=out_t[:], in0=in_t[:], in1=r[:],
                                    op=ALU.add)

        # ============================ layers ============================
        for l in range(L_LAYERS):
            ag_out = dram.tile([NCORES * DSLOT, D], bf16, tag=f"agout{l}",
                               name=f"agout{l}", addr_space="Shared")
            ag2_out = dram.tile([NCORES * 4 * GPC * C_SEEDS, MAX_N], bf16,
                                tag=f"ag2out{l}", name=f"ag2out{l}",
                                addr_space="Shared")
            # ---- per-layer weights ----
            gcnW = wp.tile([P, KC * D], f32, tag="gcnW")
            pW1 = wp.tile([P, KC * D], f32, tag="pW1")
            pW2 = wp.tile([P, KC * D], f32, tag="pW2")
            pW3 = wp.tile([P, KC * D], f32, tag="pW3")
            for kc in range(KC):
                nc.sync.dma_start(gcnW[:, kc * D:(kc + 1) * D],
                                  gcnW_d[l, kc * P:(kc + 1) * P, :])
                nc.sync.dma_start(pW1[:, kc * D:(kc + 1) * D],
                                  pW1_d[l, kc * P:(kc + 1) * P, :])
                nc.sync.dma_start(pW2[:, kc * D:(kc + 1) * D],
                                  pW2_d[l, kc * P:(kc + 1) * P, :])
                nc.sync.dma_start(pW3[:, kc * D:(kc + 1) * D],
                                  pW3_d[l, kc * P:(kc + 1) * P, :])
            eW = []
            for j in range(4):
                t = wp.tile([P, KC * D], f32, tag=f"eW{j}")
                for kc in range(KC):
                    nc.sync.dma_start(t[:, kc * D:(kc + 1) * D],
                                      eW_d[l, j, kc * P:(kc + 1) * P, :])
                eW.append(t)
            qbd = wp.tile([P, KC * P], f32, tag="qbd")
            for kc in range(KC):
                nc.sync.dma_start(qbd[:, kc * P:(kc + 1) * P],
                                  qbd_d[l, kc * P:(kc + 1) * P, :])
            gcnb = wp.tile([1, D], f32, tag="gcnb")
            nc.sync.dma_start(gcnb[:], gcnb_d[l])
            qcat = wp.tile([C_SEEDS, D], f32, tag="qcat")
            nc.sync.dma_start(qcat[:], qcat_d[l])
            pb1T = wp.tile([P, KC], f32, tag="pb1T")
            nc.sync.dma_start(pb1T[:], pb1T_d[l])
            pb2 = wp.tile([1, D], f32, tag="pb2")
            nc.sync.dma_start(pb2[:], pb2_d[l])
            pb3 = wp.tile([1, D], f32, tag="pb3")
            nc.sync.dma_start(pb3[:], pb3_d[l])
            eb0 = wp.tile([1, D], f32, tag="eb0")
            nc.sync.dma_start(eb0[:], eb0_d[l])
            eb0T = wp.tile([P, KC], f32, tag="eb0T")
            nc.sync.dma_start(eb0T[:], eb0T_d[l])
            eb1T = wp.tile([P, KC], f32, tag="eb1T")
            nc.sync.dma_start(eb1T[:], eb1T_d[l])
            eb2 = wp.tile([1, D], f32, tag="eb2")
            nc.sync.dma_start(eb2[:], eb2_d[l])
            eb3 = wp.tile([1, D], f32, tag="eb3")
            nc.sync.dma_start(eb3[:], eb3_d[l])
            plng = [[wp.tile([P, D], f32, tag=f"pln{i}{j}",
                             name=f"pln{i}{j}") for j in range(2)]
                    for i in range(2)]
            elng = [[wp.tile([P, D], f32, tag=f"eln{i}{j}",
                             name=f"eln{i}{j}") for j in range(2)]
                    for i in range(2)]
            for i in range(2):
                for j in range(2):
                    nc.sync.dma_start(plng[i][j][:], pln_d[l, i, j])
                    nc.sync.dma_start(elng[i][j][:], eln_d[l, i, j])

            # ---- 1. xw = x @ W (bf16) -> ag_in ----
            for b in range(NBLK):
                g, nt = b // NT, b % NT
                ps = psA.tile([P, D], f32, tag="psA")
                for kc in range(KC):
                    nc.tensor.matmul(
                        ps[:], lhsT=xT[g][kc][:, nt * P:(nt + 1) * P],
                        rhs=gcnW[:, kc * D:(kc + 1) * D],
                        start=(kc == 0), stop=(kc == KC - 1))
                t = xwb.tile([P, D], bf16, tag="xwb")
                nc.vector.tensor_copy(t[:], ps[:])
                nc.sync.dma_start(ag_in[b * P:(b + 1) * P, :], t[:])

            # ---- 2. AllGather ----
            nc.gpsimd.collective_compute(
                "AllGather", ALU.bypass,
                replica_groups=[list(range(NCORES))],
                ins=[ag_in[:]], outs=[ag_out[:]])

            # ---- 3. GCN aggregation per dst block ----
            xgcn = []
            gtiles = {}
            for b in range(NBLK):
                ps = psA.tile([P, D], f32, tag="psA")
                for k in range(int(nch[b])):
                    ck = int(chbase[b]) + k
                    gt = gat.tile([P, D], bf16, tag="gath")
                    nc.gpsimd.indirect_dma_start(
                        out=gt[:], out_offset=None, in_=ag_out[:],
                        in_offset=bass.IndirectOffsetOnAxis(
                            ap=esrc_t[:, ck:ck + 1], axis=0))
                    m = msel.tile([P, P], bf16, tag="msel")
                    nc.vector.tensor_scalar(
                        out=m[:], in0=iota_f[:],
                        scalar1=eldst_t[:, ck:ck + 1],
                        scalar2=ew_t[:, ck:ck + 1],
                        op0=ALU.is_equal, op1=ALU.mult)
                    nc.tensor.matmul(ps[:], lhsT=m[:], rhs=gt[:],
                                     start=(k == 0), stop=False)
                nc.tensor.matmul(ps[:], lhsT=ones_r[:], rhs=gcnb[:],
                                 start=False, stop=True)
                t = xg.tile([P, D], f32, tag="xgcn")
                nc.scalar.copy(t[:], ps[:])
                xgcn.append(t)
                if dbg and l == 0:
                    nc.sync.dma_start(dbg_xg[b * P:(b + 1) * P, :], t[:])

            # ---- 4. transpose x_gcn -> xT (overwrite xT tiles) ----
            for g in range(GPC):
                for kc in range(KC):
                    for nt in range(NT):
                        tp = psT.tile([P, P], f32, tag="tr")
                        nc.tensor.transpose(
                            tp[:], xgcn[g * NT + nt][:, kc * P:(kc + 1) * P],
                            ident[:])
                        nc.scalar.copy(xT[g][kc][:, nt * P:(nt + 1) * P], tp[:])

            # ---- 5. attention per graph: PMA -> LN/FFN -> SAB -> LN/FFN ----
            qbd2 = [att.tile([P, P], f32, tag=f"qbd2_{kc}",
                             name=f"qbd2_{kc}") for kc in range(KC)]
            for kc in range(KC):
                nc.vector.memset(qbd2[kc][:], 0.0)
            vns_f = []
            for g in range(GPC):
                kT = []
                for ft in range(KC):
                    ps = psB.tile([P, MAX_N], f32, tag="psB")
                    for kc in range(KC):
                        nc.tensor.matmul(
                            ps[:],
                            lhsT=pW1[:, kc * D + ft * P:kc * D + (ft + 1) * P],
                            rhs=xT[g][kc][:], start=(kc == 0),
                            stop=(kc == KC - 1))
                    t = att.tile([P, MAX_N], f32, tag="kT")
                    nc.vector.tensor_scalar(out=t[:], in0=ps[:],
                                            scalar1=pb1T[:, ft:ft + 1],
                                            scalar2=None, op0=ALU.add)
                    kT.append(t)
                sps = psB.tile([P, MAX_N], f32, tag="psB")
                for kc in range(KC):
                    nc.tensor.matmul(sps[:], lhsT=qbd[:, kc * P:(kc + 1) * P],
                                     rhs=kT[kc][:], start=(kc == 0), stop=False)
                ngm = att.tile([1, MAX_N], f32, tag="ngm")
                nc.sync.dma_start(ngm[:], negmask_d[0:1,
                                          g * MAX_N:(g + 1) * MAX_N])
                nc.tensor.matmul(sps[:], lhsT=ones_r[:], rhs=ngm[:],
                                 start=False, stop=True)
                nmax = att.tile([P, 1], f32, tag="nmax")
                nc.vector.tensor_reduce(nmax[:], sps[:], axis=AX.X,
                                        op=ALU.max, negate=True)
                A = att.tile([P, MAX_N], f32, tag="A")
                ssum = att.tile([P, 1], f32, tag="ssum")
                nc.scalar.activation(A[:], sps[:], AF.Exp, bias=nmax[:, 0:1],
                                     accum_out=ssum[:, 0:1])
                rinv = att.tile([P, 1], f32, tag="rinv")
                nc.vector.reciprocal(rinv[:], ssum[:])
                nc.vector.tensor_scalar(out=A[:], in0=A[:], scalar1=rinv[:, 0:1],
                                        scalar2=None, op0=ALU.mult)
                A_bf = att.tile([P, MAX_N], bf16, tag="Abf")
                nc.vector.tensor_copy(A_bf[:], A[:])
                for h in range(H_HEADS):
                    nc.sync.dma_start(
                        ag2_in[(h * GPC + g) * C_SEEDS:
                               (h * GPC + g + 1) * C_SEEDS, :],
                        A_bf[h * C_SEEDS:(h + 1) * C_SEEDS, :])
                AT = []
                for ct in range(NT):
                    tp = psT.tile([P, P], f32, tag="tr")
                    nc.tensor.transpose(tp[:], A[:, ct * P:(ct + 1) * P], ident[:])
                    t = att.tile([P, P], f32, tag="AT", bufs=4)
                    nc.scalar.copy(t[:], tp[:])
                    AT.append(t)
                aps = psC.tile([P, D], f32, tag="psC")
                for ct in range(NT):
                    nc.tensor.matmul(aps[:], lhsT=AT[ct][:],
                                     rhs=xgcn[g * NT + ct][:],
                                     start=(ct == 0), stop=(ct == NT - 1))
                AXs = att.tile([P, D], f32, tag="AXs")
                nc.scalar.copy(AXs[:], aps[:])
                AXT = []
                for kc in range(KC):
                    tp = psT.tile([P, P], f32, tag="tr")
                    nc.tensor.transpose(tp[:], AXs[:, kc * P:(kc + 1) * P],
                                        ident[:])
                    t = att.tile([P, P], f32, tag="AXT")
                    nc.scalar.copy(t[:], tp[:])
                    AXT.append(t)
                ops = psC.tile([C_SEEDS, D], f32, tag="psC")
                nc.tensor.matmul(ops[:], lhsT=ones_r[:, 0:C_SEEDS], rhs=pb2[:],
                                 start=True, stop=False)
                for h in range(H_HEADS):
                    for kc in range(KC):
                        nc.tensor.matmul(
                            ops[:, h * dh:(h + 1) * dh],
                            lhsT=AXT[kc][:, h * C_SEEDS:(h + 1) * C_SEEDS],
                            rhs=pW2[:, kc * D + h * dh:kc * D + (h + 1) * dh],
                            start=False,
                            stop=(h == H_HEADS - 1 and kc == KC - 1))
                v_a = att.tile([C_SEEDS, D], f32, tag="v_a")
                nc.vector.tensor_tensor(out=v_a[:], in0=ops[:], in1=qcat[:],
                                        op=ALU.add)
                if dbg and l == 0:
                    nc.sync.dma_start(
                        dbg_va[g * C_SEEDS:(g + 1) * C_SEEDS, :], v_a[:])
                    nc.sync.dma_start(dbg_A[g * P:(g + 1) * P, :], A[:])
                # PMA LN -> FFN -> LN
                t1 = att.tile([C_SEEDS, D], f32, tag="lnt1", bufs=1)
                ln_norm(v_a, plng[0][0], plng[0][1], t1)
                t2_ = att.tile([C_SEEDS, D], f32, tag="lnt2", bufs=1)
                ffn_block(t1, pW3, pb3, t2_)
                v_b = att.tile([C_SEEDS, D], f32, tag="v_b")
                ln_norm(t2_, plng[1][0], plng[1][1], v_b)

                # ---- SAB (within graph) ----
                vT = []
                for kc in range(KC):
                    tp = psT.tile([P, C_SEEDS], f32, tag="tr")
                    nc.tensor.transpose(tp[:], v_b[:, kc * P:(kc + 1) * P],
                                        ident[0:C_SEEDS, 0:C_SEEDS])
                    t = att.tile([P, C_SEEDS], f32, tag="vT")
                    nc.scalar.copy(t[:], tp[:])
                    vT.append(t)
                qps = psC.tile([C_SEEDS, D], f32, tag="psC")
                for kc in range(KC):
                    nc.tensor.matmul(qps[:], lhsT=vT[kc][:],
                                     rhs=eW[0][:, kc * D:(kc + 1) * D],
                                     start=(kc == 0), stop=False)
                nc.tensor.matmul(qps[:], lhsT=ones_r[:, 0:C_SEEDS], rhs=eb0[:],
                                 start=False, stop=True)
                q_s = att.tile([C_SEEDS, D], f32, tag="q_s")
                nc.scalar.copy(q_s[:], qps[:])
                qT_s, kT_s = [], []
                for ft in range(KC):
                    ps1 = psT.tile([P, C_SEEDS], f32, tag="tr")
                    ps2 = psT.tile([P, C_SEEDS], f32, tag="tr")
                    for kc in range(KC):
                        nc.tensor.matmul(
                            ps1[:],
                            lhsT=eW[0][:, kc * D + ft * P:kc * D + (ft + 1) * P],
                            rhs=vT[kc][:], start=(kc == 0), stop=(kc == KC - 1))
                        nc.tensor.matmul(
                            ps2[:],
                            lhsT=eW[1][:, kc * D + ft * P:kc * D + (ft + 1) * P],
                            rhs=vT[kc][:], start=(kc == 0), stop=(kc == KC - 1))
                    tq = att.tile([P, C_SEEDS], f32, tag="qT")
                    nc.vector.tensor_scalar(out=tq[:], in0=ps1[:],
                                            scalar1=eb0T[:, ft:ft + 1],
                                            scalar2=ISCALE,
                                            op0=ALU.add, op1=ALU.mult)
                    tk = att.tile([P, C_SEEDS], f32, tag="kTs")
                    nc.vector.tensor_scalar(out=tk[:], in0=ps2[:],
                                            scalar1=eb1T[:, ft:ft + 1],
                                            scalar2=None, op0=ALU.add)
                    qT_s.append(tq)
                    kT_s.append(tk)
                for kc in range(KC):
                    h0 = kc * 2
                    nc.vector.tensor_copy(
                        qbd2[kc][0:64, h0 * C_SEEDS:(h0 + 1) * C_SEEDS],
                        qT_s[kc][0:64, :])
                    nc.vector.tensor_copy(
                        qbd2[kc][64:128, (h0 + 1) * C_SEEDS:(h0 + 2) * C_SEEDS],
                        qT_s[kc][64:128, :])
                sps2 = psT.tile([P, C_SEEDS], f32, tag="tr")
                for kc in range(KC):
                    nc.tensor.matmul(sps2[:], lhsT=qbd2[kc][:],
                                     rhs=kT_s[kc][:],
                                     start=(kc == 0), stop=(kc == KC - 1))
                nmax2 = att.tile([P, 1], f32, tag="nmax")
                nc.vector.tensor_reduce(nmax2[:], sps2[:], axis=AX.X,
                                        op=ALU.max, negate=True)
                A2 = att.tile([P, C_SEEDS], f32, tag="A2")
                ssum2 = att.tile([P, 1], f32, tag="ssum")
                nc.scalar.activation(A2[:], sps2[:], AF.Exp,
                                     bias=nmax2[:, 0:1],
                                     accum_out=ssum2[:, 0:1])
                rinv2 = att.tile([P, 1], f32, tag="rinv")
                nc.vector.reciprocal(rinv2[:], ssum2[:])
                nc.vector.tensor_scalar(out=A2[:], in0=A2[:],
                                        scalar1=rinv2[:, 0:1],
                                        scalar2=None, op0=ALU.mult)
                tp = psT.tile([C_SEEDS, P], f32, tag="tr")
                nc.tensor.transpose(tp[:], A2[:], ident[:])
                AT2 = att.tile([C_SEEDS, P], f32, tag="AT2")
                nc.scalar.copy(AT2[:], tp[:])
                aps2 = psC.tile([P, D], f32, tag="psC")
                nc.tensor.matmul(aps2[:], lhsT=AT2[:], rhs=v_b[:],
                                 start=True, stop=True)
                AX2 = att.tile([P, D], f32, tag="AX2")
                nc.scalar.copy(AX2[:], aps2[:])
                AXT2 = []
                for kc in range(KC):
                    tp2 = psT.tile([P, P], f32, tag="tr")
                    nc.tensor.transpose(tp2[:], AX2[:, kc * P:(kc + 1) * P],
                                        ident[:])
                    t = att.tile([P, P], f32, tag="AXT")
                    nc.scalar.copy(t[:], tp2[:])
                    AXT2.append(t)
                ops2 = psC.tile([C_SEEDS, D], f32, tag="psC")
                nc.tensor.matmul(ops2[:], lhsT=ones_r[:, 0:C_SEEDS],
                                 rhs=eb2[:], start=True, stop=False)
                for h in range(H_HEADS):
                    for kc in range(KC):
                        nc.tensor.matmul(
                            ops2[:, h * dh:(h + 1) * dh],
                            lhsT=AXT2[kc][:, h * C_SEEDS:(h + 1) * C_SEEDS],
                            rhs=eW[2][:, kc * D + h * dh:kc * D + (h + 1) * dh],
                            start=False,
                            stop=(h == H_HEADS - 1 and kc == KC - 1))
                v_c = att.tile([C_SEEDS, D], f32, tag="v_c")
                nc.vector.tensor_tensor(out=v_c[:], in0=ops2[:], in1=q_s[:],
                                        op=ALU.add)
                t3 = att.tile([C_SEEDS, D], f32, tag="lnt1", bufs=1)
                ln_norm(v_c, elng[0][0], elng[0][1], t3)
                t4 = att.tile([C_SEEDS, D], f32, tag="lnt2", bufs=1)
                ffn_block(t3, eW[3], eb3, t4)
                v_f = vnsp.tile([C_SEEDS, D], bf16, tag="v_f")
                ln_norm(t4, elng[1][0], elng[1][1], v_f)
                vns_f.append(v_f)
                if dbg and l == 0:
                    vfd = att.tile([C_SEEDS, D], f32, tag="vfd", bufs=1)
                    nc.vector.tensor_copy(vfd[:], v_f[:])
                    nc.sync.dma_start(
                        dbg_vns[g * C_SEEDS:(g + 1) * C_SEEDS, :], vfd[:])

            # ---- 5b. exchange A slices for the (scrambled) comb ----
            nc.gpsimd.collective_compute(
                "AllGather", ALU.bypass,
                replica_groups=[list(range(NCORES))],
                ins=[ag2_in[:]], outs=[ag2_out[:]])

            # ---- 8. Smix gather + combT: xT[g] += (vns/H)^T @ Smix ----
            for g in range(GPC):
                gj = []
                for j in range(4):
                    t = gat.tile([C_SEEDS, MAX_N], bf16, tag="gt2", bufs=8)
                    nc.gpsimd.indirect_dma_start(
                        out=t[:], out_offset=None, in_=ag2_out[:],
                        in_offset=bass.IndirectOffsetOnAxis(
                            ap=es2_t[:, 4 * g + j:4 * g + j + 1], axis=0))
                    gj.append(t)
                sx = smp.tile([C_SEEDS, MAX_N], f32, tag="smix")
                sx2 = smp.tile([C_SEEDS, MAX_N], f32, tag="smix2")
                nc.vector.tensor_tensor(out=sx[:], in0=gj[0][:], in1=gj[1][:],
                                        op=ALU.add)
                nc.vector.tensor_tensor(out=sx2[:], in0=gj[2][:], in1=gj[3][:],
                                        op=ALU.add)
                nc.vector.tensor_tensor(out=sx[:], in0=sx[:], in1=sx2[:],
                                        op=ALU.add)
                vg = att.tile([C_SEEDS, D], f32, tag="vg")
                nc.scalar.mul(vg[:], vns_f[g][:], 1.0 / H_HEADS)
                for kc in range(KC):
                    ps = psB.tile([P, MAX_N], f32, tag="psB")
                    nc.tensor.matmul(ps[:], lhsT=vg[:, kc * P:(kc + 1) * P],
                                     rhs=sx[:], start=True, stop=True)
                    nc.vector.tensor_tensor(out=xT[g][kc][:], in0=xT[g][kc][:],
                                            in1=ps[:], op=ALU.add)
                if dbg and l == 0:
                    for kc in range(KC):
                        nc.sync.dma_start(
                            dbg_xt[kc * P:(kc + 1) * P,
                                   g * MAX_N:(g + 1) * MAX_N],
                            xT[g][kc][:])

        # ============================ head ============================
        for b in range(NBLK):
            g, nt = b // NT, b % NT
            ps = psT.tile([P, D_OUT], f32, tag="tr")
            for kc in range(KC):
                nc.tensor.matmul(ps[:], lhsT=xT[g][kc][:, nt * P:(nt + 1) * P],
                                 rhs=headW_t[:, kc * D_OUT:(kc + 1) * D_OUT],
                                 start=(kc == 0), stop=(kc == KC - 1))
            yt = att.tile([P, D_OUT], f32, tag="yt")
            nc.vector.tensor_tensor(out=yt[:], in0=ps[:], in1=headb_t[:],
                                    op=ALU.add)
            nc.sync.dma_start(y_d[b * P:(b + 1) * P, :], yt[:])

    nc.compile()
    return nc


# ----------------------------------------------------------------------------
# Entry point
# ----------------------------------------------------------------------------

def kernel(x, gcn_W, gcn_b, seeds, proj_W, proj_b, proj_ln_g, proj_ln_b,
           exch_W, exch_b, exch_ln_g, exch_ln_b, head_W, head_b,
           edge_index, batch_ids):
    global LAST_EXEC_TIME_NS
    meta, pre = _preprocess(x, edge_index, batch_ids, seeds, proj_W, proj_b)
    TCH = pre["esrc"].shape[2]

    if "nc" not in _CACHE:
        _CACHE["nc"] = _build(TCH, meta["nch"])
    nc = _CACHE["nc"]

    bcast = lambda v: np.broadcast_to(np.asarray(v, np.float32), (P, D)).copy()
    pln = np.zeros((L_LAYERS, 2, 2, P, D), np.float32)
    eln = np.zeros((L_LAYERS, 2, 2, P, D), np.float32)
    for l in range(L_LAYERS):
        for i in range(2):
            pln[l, i, 0] = bcast(proj_ln_g[l][i])
            pln[l, i, 1] = bcast(proj_ln_b[l][i])
            eln[l, i, 0] = bcast(exch_ln_g[l][i])
            eln[l, i, 1] = bcast(exch_ln_b[l][i])

    colT = lambda v: np.asarray(v, np.float32).reshape(L_LAYERS, KC, P) \
        .transpose(0, 2, 1).copy()
    pb = np.asarray(proj_b, np.float32)
    eb = np.asarray(exch_b, np.float32)
    shared = dict(
        qbd=pre["qbd"], qcat=pre["qcat"],
        gcnW=np.asarray(gcn_W, np.float32),
        gcnb=np.asarray(gcn_b, np.float32).reshape(L_LAYERS, 1, D),
        pW1=np.asarray(proj_W, np.float32)[:, 1].copy(),
        pb1T=colT(pb[:, 1]),
        pW2=np.asarray(proj_W, np.float32)[:, 2].copy(),
        pb2=pb[:, 2].reshape(L_LAYERS, 1, D).copy(),
        pW3=np.asarray(proj_W, np.float32)[:, 3].copy(),
        pb3=pb[:, 3].reshape(L_LAYERS, 1, D).copy(),
        pln=pln,
        eW=np.asarray(exch_W, np.float32),
        eb0=eb[:, 0].reshape(L_LAYERS, 1, D).copy(),
        eb0T=colT(eb[:, 0]),
        eb1T=colT(eb[:, 1]),
        eb2=eb[:, 2].reshape(L_LAYERS, 1, D).copy(),
        eb3=eb[:, 3].reshape(L_LAYERS, 1, D).copy(),
        eln=eln,
        headW=np.asarray(head_W, np.float32),
        headb=np.broadcast_to(np.asarray(head_b, np.float32),
                              (P, D_OUT)).copy(),
    )
    in_maps = []
    for c in range(NCORES):
        m = dict(shared)
        m["xT0"] = pre["xT"][c]
        m["esrc"] = pre["esrc"][c]
        m["eldst"] = pre["eldst"][c]
        m["ew"] = pre["ew"][c]
        m["negmask"] = pre["negmask"][c]
        m["es2"] = pre["es2"][c]
        in_maps.append(m)

    if os.environ.get("ANT_BENCH", "1") == "1":
        results, tmin = _run_pjrt_timed(
            nc, in_maps, NCORES,
            iters=int(os.environ.get("ANT_BENCH_ITERS", "3")))
        LAST_EXEC_TIME_NS = int(tmin * 1e9) if tmin else None
    else:
        res = run_bass_kernel_spmd(nc, in_maps, list(range(NCORES)))
        results = res.results
        LAST_EXEC_TIME_NS = res.exec_time_ns

    gcore, dslot = meta["gcore"], meta["dslot"]
    y = np.zeros((N_NODES, D_OUT), np.float32)
    for c in range(NCORES):
        idx = np.where(gcore == c)[0]
        y[idx] = results[c]["y"][dslot[idx]]
    return y


# ----------------------------------------------------------------------------
# Timed PJRT runner (jit once, time warm iterations)
# ----------------------------------------------------------------------------

def _run_pjrt_timed(nc, in_maps, n_cores, iters=2):
    import time as _time
    import jax
    from jax.experimental.shard_map import shard_map
    from jax.sharding import Mesh, NamedSharding, PartitionSpec
    from concourse import bass2jax as B
    from concourse import mybir as mb

    B.install_neuronx_cc_hook()
    partition_name = (nc.partition_id_tensor.name
                      if nc.partition_id_tensor else None)
    in_names, out_names, out_avals, zero_shapes = [], [], [], []
    for alloc in nc.m.functions[0].allocations:
        if not isinstance(alloc, mb.MemoryLocationSet):
            continue
        name = alloc.memorylocations[0].name
        if alloc.kind == "ExternalInput":
            if name != partition_name:
                in_names.append(name)
        elif alloc.kind == "ExternalOutput":
            shape = tuple(alloc.tensor_shape)
            dtype = mb.dt.np(alloc.dtype)
            out_names.append(name)
            out_avals.append(jax.core.ShapedArray(shape, dtype))
            zero_shapes.append((shape, dtype))
    n_params = len(in_names)
    n_outs = len(out_names)
    all_in = list(in_names) + list(out_names)
    if partition_name is not None:
        all_in.append(partition_name)
    donate = tuple(range(n_params, n_params + n_outs))

    def _body(*args):
        operands = list(args)
        if partition_name is not None:
            operands.append(B.partition_id_tensor())
        return tuple(B._bass_exec_p.bind(
            *operands, out_avals=tuple(out_avals), in_names=tuple(all_in),
            out_names=tuple(out_names), lowering_input_output_aliases=(),
            sim_require_finite=True, sim_require_nnan=True, nc=nc))

    devices = jax.devices()[:n_cores]
    mesh = Mesh(np.asarray(devices), ("core",))
    sh = NamedSharding(mesh, PartitionSpec("core"))
    in_specs = (PartitionSpec("core"),) * (n_params + n_outs)
    out_specs = (PartitionSpec("core"),) * n_outs
    sharded = jax.jit(
        shard_map(_body, mesh=mesh, in_specs=in_specs, out_specs=out_specs,
                  check_rep=False),
        donate_argnums=donate, keep_unused=True)

    dev_in = [
        jax.device_put(
            np.concatenate([np.asarray(in_maps[c][n]) for c in range(n_cores)],
                           axis=0), sh)
        for n in in_names
    ]

    def zeros():
        return [jax.device_put(
            np.zeros((n_cores * s[0], *s[1:]), d), sh)
            for s, d in zero_shapes]

    outs = sharded(*dev_in, *zeros())
    outs = [np.asarray(o) for o in outs]
    times = []
    for _ in range(iters):
        z = zeros()
        jax.block_until_ready(z)
        t0 = _time.perf_counter()
        o2 = sharded(*dev_in, *z)
        jax.block_until_ready(o2)
        times.append(_time.perf_counter() - t0)
        del o2
    results = [
        {name: outs[i].reshape(n_cores, *zero_shapes[i][0])[c]
         for i, name in enumerate(out_names)}
        for c in range(n_cores)
    ]
    return results, (min(times) if times else None)



# revision 33
# speedup vs baseline: 21.7539x; 21.7539x over previous
"""Trainium2 Bass kernel for nn_CustomGNN (GCN + GMT pooling, 3 layers).

Sharding: data-parallel over graphs (16 graphs / core, 8 cores).

v2 design:
- All matmuls bf16 (4x faster than fp32 on the PE), fp32 PSUM accumulation.
- GCN aggregation: edges sharded by dst core, grouped by (graph, src-half,
  dst block), gathered with batched InstDMAGatherAnt (<=8 chunks = 1024 rows
  per instruction, int16 indices relative to a half-table base) + selection
  matrix on DVE + TensorE matmul accumulation.
- Attention batched per group of 4 graphs (128-partition tiles for all
  seed-level ops); PMA keys computed as [128, 1536] group tiles; SAB runs
  block-diagonal over 4 graphs with per-head [128,128] score matmuls.
- comb exchange: A slices AllGathered (bf16); each graph's 4 mix slices are
  contiguous 128 rows in the gathered buffer -> one direct DMA per graph,
  summed via a 0.25-blockdiag matmul, no indirect DMAs.
- Reported time: device-side NEFF execution time from the NTFF profile
  (falls back to wall-clock min if profiling is unavailable).
"""
import os
import sys
import types
import numpy as np
import ml_dtypes
from contextlib import ExitStack

import concourse.bass as bass
import concourse.tile as tile
from concourse import bacc, mybir
from concourse.bass_utils import run_bass_kernel_spmd
from concourse.masks import make_identity

P = 128
NCORES = 8
NUM_GRAPHS = 128
GPC = NUM_GRAPHS // NCORES      # 16 graphs per core
MAX_N = 384
NT = MAX_N // P                 # 3 node tiles per graph
DSLOT = GPC * MAX_N             # 6144 dense slots per core
NBLK = DSLOT // P               # 48 dst blocks per core
HALF = NCORES * DSLOT // 2      # 24576: gather-table half size (int16 idx)
N_NODES = 32768
D = 256
KC = D // P                     # 2 feature chunks
C_SEEDS = 32
H_HEADS = 4
L_LAYERS = 3
D_OUT = 32
NGRP = GPC // 4                 # 4 groups of 4 graphs
GW = 4 * MAX_N                  # 1536 dense cols per group
ISCALE = 1.0 / 16.0             # 1/sqrt(D)
MAXCH = 8                       # chunks per dma_gather (ring limit 1024 idxs)

f32 = mybir.dt.float32
bf16 = mybir.dt.bfloat16
i32 = mybir.dt.int32
i16 = mybir.dt.int16
AF = mybir.ActivationFunctionType
ALU = mybir.AluOpType
AX = mybir.AxisListType

LAST_EXEC_TIME_NS = None
_CACHE = {}


# ----------------------------------------------------------------------------
# Host preprocessing (index/structure only + weights-only folding)
# ----------------------------------------------------------------------------

def _preprocess(x, edge_index, batch_ids, seeds, proj_W, proj_b):
    src = np.asarray(edge_index[0]).astype(np.int64)
    dst = np.asarray(edge_index[1]).astype(np.int64)
    batch_ids = np.asarray(batch_ids).astype(np.int64)
    counts = np.bincount(batch_ids, minlength=NUM_GRAPHS)
    starts = np.cumsum(counts) - counts
    pos = np.arange(N_NODES, dtype=np.int64) - starts[batch_ids]
    gcore = batch_ids // GPC
    glocal = batch_ids % GPC
    dslot = glocal * MAX_N + pos                      # [N] slot within core
    gslot = gcore * DSLOT + dslot                     # [N] global dense slot

    deg = 1.0 + np.bincount(dst, minlength=N_NODES).astype(np.float64)
    dis = 1.0 / np.sqrt(deg)
    src_a = np.concatenate([src, np.arange(N_NODES, dtype=np.int64)])
    dst_a = np.concatenate([dst, np.arange(N_NODES, dtype=np.int64)])
    w_a = np.concatenate([(dis[src] * dis[dst]).astype(np.float32),
                          (1.0 / deg).astype(np.float32)])

    ecore = gcore[dst_a]
    per_core = []
    cnt = np.zeros((NCORES, NBLK, 2), np.int64)
    for c in range(NCORES):
        m = ecore == c
        es, ed, ew = src_a[m], dst_a[m], w_a[m]
        sd = dslot[ed]                                # dst slot in core
        sg = gslot[es]                                # src global slot
        hf = (sg >= HALF).astype(np.int64)
        blk = sd // P
        order = np.lexsort((np.arange(len(es)), hf, blk))
        sd, sg, hf, blk, ew = sd[order], sg[order], hf[order], blk[order], ew[order]
        for b in range(NBLK):
            for h in (0, 1):
                cnt[c, b, h] = ((blk == b) & (hf == h)).sum()
        per_core.append((sd, sg, hf, blk, ew))

    # per (block, half) chunk counts: max over cores (shared compiled program)
    nch = np.maximum(0, np.ceil(cnt / P).astype(np.int64).max(axis=0))  # [NBLK,2]
    # chunk order: (graph, half, block, k)
    chbase = np.zeros((NBLK, 2), np.int64)
    chunk_list = []   # (block, half)
    tc_i = 0
    gathers = []      # (chunk_start, nchunks, half, graph)
    for g in range(GPC):
        for h in (0, 1):
            g_start = tc_i
            for b in range(g * NT, (g + 1) * NT):
                chbase[b, h] = tc_i
                for _ in range(int(nch[b, h])):
                    chunk_list.append((b, h))
                    tc_i += 1
            m = tc_i - g_start
            # split into <=MAXCH pieces
            s = g_start
            while m > 0:
                n = min(MAXCH, m)
                gathers.append((s, n, h, g))
                s += n
                m -= n
    TCH = tc_i

    esrc16 = np.zeros((NCORES, TCH * P), np.int16)
    eldst = np.zeros((NCORES, P, TCH), np.float32)
    ew_arr = np.zeros((NCORES, P, TCH), np.float32)
    for c in range(NCORES):
        sd, sg, hf, blk, ew = per_core[c]
        # position within (block, half)
        for b in range(NBLK):
            for h in (0, 1):
                m = (blk == b) & (hf == h)
                n = int(m.sum())
                if n == 0:
                    continue
                k = np.arange(n)
                ck = chbase[b, h] + k // P
                row = k % P
                esrc16[c, ck * P + row] = (sg[m] - h * HALF).astype(np.int16)
                eldst[c, row, ck] = (sd[m] % P).astype(np.float32)
                ew_arr[c, row, ck] = ew[m]

    # wrapped idx layout per gather: idx i -> [i%16, i//16], replicated x8
    idxw = np.zeros((NCORES, P, TCH * 8), np.int16)
    for c in range(NCORES):
        for (cs, n, h, g) in gathers:
            fl = esrc16[c, cs * P:(cs + n) * P]
            w = fl.reshape(n * 8, 16).T                  # [16, n*8]
            idxw[c, :, cs * 8:(cs + n) * 8] = np.tile(w, (8, 1))

    negmask = np.zeros((NCORES, 1, DSLOT), np.float32)
    cnts = counts.reshape(NCORES, GPC)
    sl = np.arange(DSLOT)
    for c in range(NCORES):
        real = sl % MAX_N < cnts[c][sl // MAX_N]
        negmask[c, 0, ~real] = -1e9

    xT = np.zeros((NCORES, D, DSLOT), np.float32)
    xx = np.asarray(x)
    for c in range(NCORES):
        idx = np.where(gcore == c)[0]
        xT[c][:, dslot[idx]] = xx[idx].T

    # host-folded PMA query (weights-only)
    qbd = np.zeros((L_LAYERS, D, P), np.float32)
    qcat4 = np.zeros((L_LAYERS, P, D), np.float32)
    dh = D // H_HEADS
    for l in range(L_LAYERS):
        qc = np.asarray(seeds[l]) @ np.asarray(proj_W[l][0]) + np.asarray(proj_b[l][0])
        qcat4[l] = np.tile(qc, (4, 1))
        for h in range(H_HEADS):
            qbd[l, h * dh:(h + 1) * dh, h * C_SEEDS:(h + 1) * C_SEEDS] = \
                qc[:, h * dh:(h + 1) * dh].T * ISCALE

    # smix: graph g needs 128 contiguous rows of ag2_out (its 4 A-slices,
    # per-core row base). Two dma_gathers of 8 graphs x 128 rows each.
    idxw2 = np.zeros((NCORES, P, 2 * 64), np.int16)
    for c in range(NCORES):
        for half in (0, 1):
            fl = np.zeros(8 * P, np.int16)
            for gl in range(8):
                g = 8 * half + gl
                b2 = GPC * c + g
                mflat = 4 * b2
                h, b = mflat // NUM_GRAPHS, mflat % NUM_GRAPHS
                r0 = (b // GPC) * 2048 + (h * GPC + (b % GPC)) * C_SEEDS
                fl[gl * P:(gl + 1) * P] = r0 + np.arange(P)
            w = fl.reshape(64, 16).T
            idxw2[c, :, half * 64:(half + 1) * 64] = np.tile(w, (8, 1))

    # SAB block-diagonal mask [128,128]; smix selection matrices [4,128,128]
    ii = np.arange(P)
    sabmask = np.where((ii[:, None] // C_SEEDS) == (ii[None, :] // C_SEEDS),
                       0.0, -1e9).astype(np.float32)
    selmix = np.zeros((4, P, P), np.float32)
    for gg in range(4):
        selmix[gg, ii, gg * C_SEEDS + ii % C_SEEDS] = 1.0 / H_HEADS

    meta = dict(nch=nch, gcore=gcore, dslot=dslot, gathers=gathers,
                chbase=chbase, TCH=TCH)
    return meta, dict(idxw=idxw, idxw2=idxw2, eldst=eldst, ew=ew_arr,
                      negmask=negmask, xT=xT, qbd=qbd, qcat4=qcat4,
                      sabmask=sabmask, selmix=selmix)


# ----------------------------------------------------------------------------
# Device kernel
# ----------------------------------------------------------------------------

def _build(meta):
    TCH = meta["TCH"]
    nch = meta["nch"]
    chbase = meta["chbase"]
    gathers = meta["gathers"]
    dh = D // H_HEADS

    nc = bacc.Bacc("TRN2", target_bir_lowering=False, debug=False,
                   num_devices=NCORES, num_swdge_queues=2)

    def din(name, shape, dt=f32):
        return nc.dram_tensor(name, shape, dt, kind="ExternalInput")

    xT0_d = din("xT0", [D, DSLOT], bf16)
    idxw_d = din("idxw", [P, TCH * 8], i16)
    idxw2_d = din("idxw2", [P, 2 * 64], i16)
    eldst_d = din("eldst", [P, TCH])
    ew_d = din("ew", [P, TCH])
    negmask_d = din("negmask", [1, DSLOT], bf16)
    qbd_d = din("qbd", [L_LAYERS, D, P], bf16)
    qcat4_d = din("qcat4", [L_LAYERS, P, D])
    sabmask_d = din("sabmask", [P, P])
    selmix_d = din("selmix", [4, P, P], bf16)
    gcnW_d = din("gcnW", [L_LAYERS, D, D], bf16)
    gcnb_d = din("gcnb", [L_LAYERS, 1, D], bf16)
    pW1_d = din("pW1", [L_LAYERS, D, D], bf16)
    pb1T_d = din("pb1T", [L_LAYERS, P, KC])
    pW2_d = din("pW2", [L_LAYERS, D, D], bf16)
    pb2_d = din("pb2", [L_LAYERS, 1, D], bf16)
    pW3_d = din("pW3", [L_LAYERS, D, D], bf16)
    pb3_d = din("pb3", [L_LAYERS, 1, D], bf16)
    pln_d = din("pln", [L_LAYERS, 2, 2, P, D])
    eW_d = din("eW", [L_LAYERS, 4, D, D], bf16)
    eb0_d = din("eb0", [L_LAYERS, 1, D], bf16)
    eb0T_d = din("eb0T", [L_LAYERS, P, KC])
    eb1T_d = din("eb1T", [L_LAYERS, P, KC])
    eb2_d = din("eb2", [L_LAYERS, 1, D], bf16)
    eb3_d = din("eb3", [L_LAYERS, 1, D], bf16)
    eln_d = din("eln", [L_LAYERS, 2, 2, P, D])
    headW_d = din("headW", [D, D_OUT], bf16)
    headb_d = din("headb", [P, D_OUT])

    y_d = nc.dram_tensor("y", [DSLOT, D_OUT], f32, kind="ExternalOutput")

    # per-graph gather plan and chunk->(gather,local) map
    gathers_of = [[] for _ in range(GPC)]
    for gi, (cs, n, h, g) in enumerate(gathers):
        gathers_of[g].append((gi, cs, n, h))
    chunk2g = {}
    for gi, (cs, n, h, g) in enumerate(gathers):
        for k in range(n):
            chunk2g[cs + k] = (gi, k)

    with tile.TileContext(nc) as tc, ExitStack() as ctx:
        cst = ctx.enter_context(tc.tile_pool(name="cst", bufs=1))
        wp = ctx.enter_context(tc.tile_pool(name="wp", bufs=1))
        xTp = ctx.enter_context(tc.tile_pool(name="xTp", bufs=1))
        xg = ctx.enter_context(tc.tile_pool(name="xg", bufs=24))
        xwb = ctx.enter_context(tc.tile_pool(name="xwb", bufs=3))
        gat = ctx.enter_context(tc.tile_pool(name="gat", bufs=6))
        msel = ctx.enter_context(tc.tile_pool(name="msel", bufs=6))
        att = ctx.enter_context(tc.tile_pool(name="att", bufs=2))
        ktp = ctx.enter_context(tc.tile_pool(name="ktp", bufs=2))
        smp = ctx.enter_context(tc.tile_pool(name="smp", bufs=2))
        vnsp = ctx.enter_context(tc.tile_pool(name="vnsp", bufs=NGRP))
        dram = ctx.enter_context(tc.tile_pool(name="dram", bufs=1, space="DRAM"))
        psA = ctx.enter_context(tc.tile_pool(name="psA", bufs=2, space="PSUM"))
        psB = ctx.enter_context(tc.tile_pool(name="psB", bufs=2, space="PSUM"))
        psC = ctx.enter_context(tc.tile_pool(name="psC", bufs=2, space="PSUM"))
        psT = ctx.enter_context(tc.tile_pool(name="psT", bufs=2, space="PSUM"))

        ag_in = dram.tile([DSLOT, D], bf16, tag="agin")
        ag2_in = dram.tile([4 * GPC * C_SEEDS, MAX_N], bf16, tag="ag2in")

        # ---- constants ----
        ident = cst.tile([P, P], bf16, tag="ident")
        make_identity(nc, ident[:])
        ident_f = cst.tile([P, P], f32, tag="identf")
        make_identity(nc, ident_f[:])
        iota_i = cst.tile([P, P], i32, tag="ioti")
        nc.gpsimd.iota(iota_i[:], pattern=[[1, P]], base=0, channel_multiplier=0)
        iota_f = cst.tile([P, P], f32, tag="iotf")
        nc.vector.tensor_copy(iota_f[:], iota_i[:])
        ones_r = cst.tile([1, P], bf16, tag="ones")
        nc.vector.memset(ones_r[:], 1.0)
        eps_t = cst.tile([P, 1], f32, tag="eps")
        nc.vector.memset(eps_t[:], 1e-5)

        idxw_t = cst.tile([P, TCH * 8], i16, tag="idxw")
        nc.sync.dma_start(idxw_t[:], idxw_d[:])
        idxw2_t = cst.tile([P, 2 * 64], i16, tag="idxw2")
        nc.sync.dma_start(idxw2_t[:], idxw2_d[:])
        eldst_t = cst.tile([P, TCH], f32, tag="eldst")
        nc.sync.dma_start(eldst_t[:], eldst_d[:])
        ew_t = cst.tile([P, TCH], f32, tag="ew")
        nc.sync.dma_start(ew_t[:], ew_d[:])
        ngm_t = cst.tile([1, DSLOT], bf16, tag="ngm")
        nc.sync.dma_start(ngm_t[:], negmask_d[:])
        sabm_t = cst.tile([P, P], f32, tag="sabm")
        nc.sync.dma_start(sabm_t[:], sabmask_d[:])
        selmix_t = []
        for gg in range(4):
            tb = cst.tile([P, P], bf16, tag=f"selm{gg}")
            nc.sync.dma_start(tb[:], selmix_d[gg])
            selmix_t.append(tb)
        headW_t = cst.tile([P, KC * D_OUT], bf16, tag="headW")
        for kc in range(KC):
            nc.sync.dma_start(headW_t[:, kc * D_OUT:(kc + 1) * D_OUT],
                              headW_d[kc * P:(kc + 1) * P, :])
        headb_t = cst.tile([P, D_OUT], f32, tag="headb")
        nc.sync.dma_start(headb_t[:], headb_d[:])

        # persistent xT group tiles [NGRP][KC] of [128, GW] bf16
        xT4 = [[xTp.tile([P, GW], bf16, tag=f"xT_{G}_{kc}", name=f"xT_{G}_{kc}")
                for kc in range(KC)] for G in range(NGRP)]
        for G in range(NGRP):
            for kc in range(KC):
                nc.sync.dma_start(
                    xT4[G][kc][:],
                    xT0_d[kc * P:(kc + 1) * P, G * GW:(G + 1) * GW])

        def ln_norm(x_t, g_bc, b_bc, out_t):
            # LayerNorm over feature dim on [128, 256]
            s1 = att.tile([P, 1], f32, tag="ln_s1")
            nc.vector.tensor_reduce(s1[:], x_t[:], axis=AX.X, op=ALU.add)
            m = att.tile([P, 1], f32, tag="ln_m")
            nc.scalar.mul(m[:], s1[:], 1.0 / D)
            xm = att.tile([P, D], f32, tag="ln_xm")
            nc.vector.tensor_scalar(out=xm[:], in0=x_t[:], scalar1=m[:, 0:1],
                                    scalar2=None, op0=ALU.subtract)
            sq = att.tile([P, D], f32, tag="ln_sq", bufs=1)
            vs = att.tile([P, 1], f32, tag="ln_vs")
            nc.scalar.activation(sq[:], xm[:], AF.Square, accum_out=vs[:, 0:1])
            sd = att.tile([P, 1], f32, tag="ln_sd")
            nc.scalar.activation(sd[:], vs[:], AF.Sqrt, bias=eps_t[:, 0:1],
                                 scale=1.0 / D)
            rsd = att.tile([P, 1], f32, tag="ln_rsd")
            nc.vector.reciprocal(rsd[:], sd[:])
            nc.vector.tensor_scalar(out=xm[:], in0=xm[:], scalar1=rsd[:, 0:1],
                                    scalar2=None, op0=ALU.mult)
            nc.vector.tensor_tensor(out=xm[:], in0=xm[:], in1=g_bc[:],
                                    op=ALU.mult)
            nc.vector.tensor_tensor(out=out_t[:], in0=xm[:], in1=b_bc[:],
                                    op=ALU.add)

        def ffn_block(in_t, W_t, b_row, out_t):
            # out = in + relu(in @ W + b) on [128, 256]
            tT = []
            for kc in range(KC):
                tp = psT.tile([P, P], f32, tag="tr")
                nc.tensor.transpose(tp[:], in_t[:, kc * P:(kc + 1) * P],
                                    ident_f[:])
                ts_ = att.tile([P, P], bf16, tag="ffn_tT")
                nc.scalar.copy(ts_[:], tp[:])
                tT.append(ts_)
            ps = psC.tile([P, D], f32, tag="psC")
            for kc in range(KC):
                nc.tensor.matmul(ps[:], lhsT=tT[kc][:],
                                 rhs=W_t[:, kc * D:(kc + 1) * D],
                                 start=(kc == 0), stop=False)
            nc.tensor.matmul(ps[:], lhsT=ones_r[:], rhs=b_row[:],
                             start=False, stop=True)
            r = att.tile([P, D], f32, tag="ffn_r", bufs=1)
            nc.scalar.activation(r[:], ps[:], AF.Relu)
            nc.vector.tensor_tensor(out=out_t[:], in0=in_t[:], in1=r[:],
                                    op=ALU.add)

        # ============================ layers ============================
        for l in range(L_LAYERS):
            ag_out = dram.tile([NCORES * DSLOT, D], bf16, tag=f"agout{l}",
                               name=f"agout{l}", addr_space="Shared")
            ag2_out = dram.tile([NCORES * 4 * GPC * C_SEEDS, MAX_N], bf16,
                                tag=f"ag2out{l}", name=f"ag2out{l}",
                                addr_space="Shared")
            # ---- per-layer weights (host pre-converted to bf16) ----
            def wload(dsrc, cols=D, dt=bf16, tag=None):
                t = wp.tile([P, KC * cols], dt, tag=tag)
                for kc in range(KC):
                    nc.sync.dma_start(t[:, kc * cols:(kc + 1) * cols],
                                      dsrc[kc * P:(kc + 1) * P, :])
                return t

            gcnW = wload(gcnW_d[l], tag="gcnW")
            pW1 = wload(pW1_d[l], tag="pW1")
            pW2 = wload(pW2_d[l], tag="pW2")
            pW3 = wload(pW3_d[l], tag="pW3")
            eW = [wload(eW_d[l, j], tag=f"eW{j}") for j in range(4)]
            qbd = wload(qbd_d[l], cols=P, tag="qbd")

            def rload(dsrc, shape, tag, dt=bf16):
                t = wp.tile(shape, dt, tag=tag)
                nc.sync.dma_start(t[:], dsrc)
                return t

            gcnb = rload(gcnb_d[l], [1, D], "gcnb")
            qcat4 = rload(qcat4_d[l], [P, D], "qcat4", dt=f32)
            pb1T = rload(pb1T_d[l], [P, KC], "pb1T", dt=f32)
            pb2 = rload(pb2_d[l], [1, D], "pb2")
            pb3 = rload(pb3_d[l], [1, D], "pb3")
            eb0 = rload(eb0_d[l], [1, D], "eb0")
            eb0T = rload(eb0T_d[l], [P, KC], "eb0T", dt=f32)
            eb1T = rload(eb1T_d[l], [P, KC], "eb1T", dt=f32)
            eb2 = rload(eb2_d[l], [1, D], "eb2")
            eb3 = rload(eb3_d[l], [1, D], "eb3")
            plng = [[rload(pln_d[l, i, j], [P, D], f"pln{i}{j}", dt=f32)
                     for j in range(2)] for i in range(2)]
            elng = [[rload(eln_d[l, i, j], [P, D], f"eln{i}{j}", dt=f32)
                     for j in range(2)] for i in range(2)]

            # ---- 1. xw = x @ W (bf16) -> ag_in ----
            for b in range(NBLK):
                G, rest = b // (4 * NT), b % (4 * NT)
                ps = psA.tile([P, D], f32, tag="psA")
                for kc in range(KC):
                    nc.tensor.matmul(
                        ps[:], lhsT=xT4[G][kc][:, rest * P:(rest + 1) * P],
                        rhs=gcnW[:, kc * D:(kc + 1) * D],
                        start=(kc == 0), stop=(kc == KC - 1))
                t = xwb.tile([P, D], bf16, tag="xwb")
                nc.vector.tensor_copy(t[:], ps[:])
                nc.sync.dma_start(ag_in[b * P:(b + 1) * P, :], t[:])

            # ---- 2. AllGather xw ----
            nc.gpsimd.collective_compute(
                "AllGather", ALU.bypass,
                replica_groups=[list(range(NCORES))],
                ins=[ag_in[:]], outs=[ag_out[:]])

            # ============ per group: aggregation + attention ============
            vf4 = [None] * NGRP     # [NGRP] bf16 [128,256] virtual nodes
            xgcn = [None] * NBLK
            for G in range(NGRP):
                # ---- 3. GCN aggregation for the 4 graphs of this group ----
                for gg in range(4):
                    g = 4 * G + gg
                    gts = {}
                    for (gi, cs, n, h) in gathers_of[g]:
                        gt = gat.tile([P, MAXCH * D], bf16, tag="gt")
                        nc.gpsimd.dma_gather(
                            gt[:, :n * D].rearrange("p (c d) -> p c d", d=D),
                            ag_out[h * HALF:, :], idxw_t[:, cs * 8:(cs + n) * 8],
                            n * P, n * P, D, queue_num=gi % 2)
                        gts[gi] = gt
                    for b in range(g * NT, (g + 1) * NT):
                        ps = psA.tile([P, D], f32, tag="psA")
                        first = True
                        for h in (0, 1):
                            for k in range(int(nch[b, h])):
                                ck = int(chbase[b, h]) + k
                                gi, lc = chunk2g[ck]
                                m = msel.tile([P, P], bf16, tag="msel")
                                nc.vector.tensor_scalar(
                                    out=m[:], in0=iota_f[:],
                                    scalar1=eldst_t[:, ck:ck + 1],
                                    scalar2=ew_t[:, ck:ck + 1],
                                    op0=ALU.is_equal, op1=ALU.mult)
                                nc.tensor.matmul(
                                    ps[:], lhsT=m[:],
                                    rhs=gts[gi][:, lc * D:(lc + 1) * D],
                                    start=first, stop=False)
                                first = False
                        nc.tensor.matmul(ps[:], lhsT=ones_r[:], rhs=gcnb[:],
                                         start=first, stop=True)
                        t = xg.tile([P, D], bf16, tag="xgcn")
                        nc.scalar.copy(t[:], ps[:])
                        xgcn[b] = t
                        # ---- 4. transpose into xT4 ----
                        nt = b % NT
                        for kc in range(KC):
                            tp = psT.tile([P, P], bf16, tag="tr")
                            nc.tensor.transpose(tp[:], t[:, kc * P:(kc + 1) * P],
                                                ident[:])
                            nc.scalar.copy(
                                xT4[G][kc][:, (gg * NT + nt) * P:
                                           (gg * NT + nt + 1) * P], tp[:])

                # ---- 5. PMA for the group ----
                # keys kT[ft] [128, GW] bf16
                kT = []
                for ft in range(KC):
                    t = ktp.tile([P, GW], bf16, tag=f"kT{ft}")
                    for cc in range(GW // 512):
                        ps = psB.tile([P, 512], f32, tag="psB")
                        for kc in range(KC):
                            nc.tensor.matmul(
                                ps[:],
                                lhsT=pW1[:, kc * D + ft * P:kc * D + (ft + 1) * P],
                                rhs=xT4[G][kc][:, cc * 512:(cc + 1) * 512],
                                start=(kc == 0), stop=(kc == KC - 1))
                        nc.vector.tensor_scalar(
                            out=t[:, cc * 512:(cc + 1) * 512], in0=ps[:],
                            scalar1=pb1T[:, ft:ft + 1], scalar2=None,
                            op0=ALU.add)
                    kT.append(t)
                # AXT4h[kc] columns: h-major, then (g, c) — so the ops-stage
                # stationary is a contiguous [128, 128] slice per (h, kc)
                AXT4h = [smp.tile([P, 4 * P], bf16, tag=f"AXT4_{kc}",
                                  name=f"AXT4_{kc}")
                         for kc in range(KC)]
                for gg in range(4):
                    g = 4 * G + gg
                    sps = psB.tile([P, MAX_N], f32, tag="psB")
                    for ft in range(KC):
                        nc.tensor.matmul(
                            sps[:], lhsT=qbd[:, ft * P:(ft + 1) * P],
                            rhs=kT[ft][:, gg * MAX_N:(gg + 1) * MAX_N],
                            start=(ft == 0), stop=False)
                    nc.tensor.matmul(
                        sps[:], lhsT=ones_r[:],
                        rhs=ngm_t[0:1, g * MAX_N:(g + 1) * MAX_N],
                        start=False, stop=True)
                    nmax = att.tile([P, 1], f32, tag="nmax")
                    nc.vector.tensor_reduce(nmax[:], sps[:], axis=AX.X,
                                            op=ALU.max, negate=True)
                    A = att.tile([P, MAX_N], f32, tag="A")
                    ssum = att.tile([P, 1], f32, tag="ssum")
                    nc.scalar.activation(A[:], sps[:], AF.Exp, bias=nmax[:, 0:1],
                                         accum_out=ssum[:, 0:1])
                    rinv = att.tile([P, 1], f32, tag="rinv")
                    nc.vector.reciprocal(rinv[:], ssum[:])
                    A_bf = att.tile([P, MAX_N], bf16, tag="Abf")
                    nc.vector.tensor_scalar(out=A_bf[:], in0=A[:],
                                            scalar1=rinv[:, 0:1],
                                            scalar2=None, op0=ALU.mult)
                    for h in range(H_HEADS):
                        nc.sync.dma_start(
                            ag2_in[(h * GPC + g) * C_SEEDS:
                                   (h * GPC + g + 1) * C_SEEDS, :],
                            A_bf[h * C_SEEDS:(h + 1) * C_SEEDS, :])
                    aps = psC.tile([P, D], f32, tag="psC")
                    for ct in range(NT):
                        tp = psT.tile([P, P], bf16, tag="tr")
                        nc.tensor.transpose(tp[:], A_bf[:, ct * P:(ct + 1) * P],
                                            ident[:])
                        at = att.tile([P, P], bf16, tag="AT", bufs=4)
                        nc.scalar.copy(at[:], tp[:])
                        nc.tensor.matmul(aps[:], lhsT=at[:],
                                         rhs=xgcn[g * NT + ct][:],
                                         start=(ct == 0), stop=(ct == NT - 1))
                    AXs = att.tile([P, D], bf16, tag="AXs")
                    nc.scalar.copy(AXs[:], aps[:])
                    for kc in range(KC):
                        tp = psT.tile([P, P], bf16, tag="tr")
                        nc.tensor.transpose(tp[:], AXs[:, kc * P:(kc + 1) * P],
                                            ident[:])
                        for h in range(H_HEADS):
                            nc.scalar.copy(
                                AXT4h[kc][:, h * P + gg * C_SEEDS:
                                          h * P + (gg + 1) * C_SEEDS],
                                tp[:, h * C_SEEDS:(h + 1) * C_SEEDS])
                # ops: out4 = headmix(AXT4h) @ pW2 + pb2  -> [128, 256]
                ops = psC.tile([P, D], f32, tag="psC")
                nc.tensor.matmul(ops[:], lhsT=ones_r[:], rhs=pb2[:],
                                 start=True, stop=False)
                for h in range(H_HEADS):
                    for kc in range(KC):
                        nc.tensor.matmul(
                            ops[:, h * dh:(h + 1) * dh],
                            lhsT=AXT4h[kc][:, h * P:(h + 1) * P],
                            rhs=pW2[:, kc * D + h * dh:kc * D + (h + 1) * dh],
                            start=False,
                            stop=(h == H_HEADS - 1 and kc == KC - 1))
                v_a = att.tile([P, D], f32, tag="v_a")
                nc.vector.tensor_tensor(out=v_a[:], in0=ops[:], in1=qcat4[:],
                                        op=ALU.add)
                t1 = att.tile([P, D], f32, tag="lnt1", bufs=1)
                ln_norm(v_a, plng[0][0], plng[0][1], t1)
                t2_ = att.tile([P, D], f32, tag="lnt2", bufs=1)
                ffn_block(t1, pW3, pb3, t2_)
                v_b = att.tile([P, D], f32, tag="v_b")
                ln_norm(t2_, plng[1][0], plng[1][1], v_b)

                # ---- 6. SAB (block-diagonal over 4 graphs) ----
                v_bb = att.tile([P, D], bf16, tag="v_bb")
                nc.vector.tensor_copy(v_bb[:], v_b[:])
                vT4 = []
                for kc in range(KC):
                    tp = psT.tile([P, P], bf16, tag="tr")
                    nc.tensor.transpose(tp[:], v_bb[:, kc * P:(kc + 1) * P],
                                        ident[:])
                    t = att.tile([P, P], bf16, tag="vT")
                    nc.scalar.copy(t[:], tp[:])
                    vT4.append(t)
                qps = psC.tile([P, D], f32, tag="psC")
                for kc in range(KC):
                    nc.tensor.matmul(qps[:], lhsT=vT4[kc][:],
                                     rhs=eW[0][:, kc * D:(kc + 1) * D],
                                     start=(kc == 0), stop=False)
                nc.tensor.matmul(qps[:], lhsT=ones_r[:], rhs=eb0[:],
                                 start=False, stop=True)
                q_s = att.tile([P, D], f32, tag="q_s")
                nc.scalar.copy(q_s[:], qps[:])
                qT4, kT4 = [], []
                for ft in range(KC):
                    ps1 = psT.tile([P, P], f32, tag="tr")
                    ps2 = psT.tile([P, P], f32, tag="tr")
                    for kc in range(KC):
                        nc.tensor.matmul(
                            ps1[:],
                            lhsT=eW[0][:, kc * D + ft * P:kc * D + (ft + 1) * P],
                            rhs=vT4[kc][:], start=(kc == 0), stop=(kc == KC - 1))
                        nc.tensor.matmul(
                            ps2[:],
                            lhsT=eW[1][:, kc * D + ft * P:kc * D + (ft + 1) * P],
                            rhs=vT4[kc][:], start=(kc == 0), stop=(kc == KC - 1))
                    tq = att.tile([P, P], bf16, tag="qT")
                    nc.vector.tensor_scalar(out=tq[:], in0=ps1[:],
                                            scalar1=eb0T[:, ft:ft + 1],
                                            scalar2=ISCALE,
                                            op0=ALU.add, op1=ALU.mult)
                    tk = att.tile([P, P], bf16, tag="kTs")
                    nc.vector.tensor_scalar(out=tk[:], in0=ps2[:],
                                            scalar1=eb1T[:, ft:ft + 1],
                                            scalar2=None, op0=ALU.add)
                    qT4.append(tq)
                    kT4.append(tk)
                ops2 = psB.tile([P, D], f32, tag="psB")
                nc.tensor.matmul(ops2[:], lhsT=ones_r[:], rhs=eb2[:],
                                 start=True, stop=False)
                for h in range(H_HEADS):
                    ft, r0 = h // 2, (h % 2) * dh
                    sps2 = psT.tile([P, P], f32, tag="tr")
                    nc.tensor.matmul(sps2[:], lhsT=qT4[ft][r0:r0 + dh, :],
                                     rhs=kT4[ft][r0:r0 + dh, :],
                                     start=True, stop=True)
                    nc.vector.tensor_tensor(out=sps2[:], in0=sps2[:],
                                            in1=sabm_t[:], op=ALU.add)
                    nmax2 = att.tile([P, 1], f32, tag="nmax")
                    nc.vector.tensor_reduce(nmax2[:], sps2[:], axis=AX.X,
                                            op=ALU.max, negate=True)
                    A2 = att.tile([P, P], f32, tag="A2")
                    ssum2 = att.tile([P, 1], f32, tag="ssum")
                    nc.scalar.activation(A2[:], sps2[:], AF.Exp,
                                         bias=nmax2[:, 0:1],
                                         accum_out=ssum2[:, 0:1])
                    rinv2 = att.tile([P, 1], f32, tag="rinv")
                    nc.vector.reciprocal(rinv2[:], ssum2[:])
                    A2b = att.tile([P, P], bf16, tag="A2b")
                    nc.vector.tensor_scalar(out=A2b[:], in0=A2[:],
                                            scalar1=rinv2[:, 0:1],
                                            scalar2=None, op0=ALU.mult)
                    tp = psT.tile([P, P], bf16, tag="tr")
                    nc.tensor.transpose(tp[:], A2b[:], ident[:])
                    A2T = att.tile([P, P], bf16, tag="A2T")
                    nc.scalar.copy(A2T[:], tp[:])
                    # AV_h = A2 @ v_b  (full width), then @ eW2 head cols
                    AV = psC.tile([P, D], f32, tag="psC")
                    nc.tensor.matmul(AV[:], lhsT=A2T[:], rhs=v_bb[:],
                                     start=True, stop=True)
                    avs = att.tile([P, D], bf16, tag="avs", bufs=2)
                    nc.scalar.copy(avs[:], AV[:])
                    AXT2 = []
                    for kc in range(KC):
                        tp2 = psT.tile([P, P], bf16, tag="tr")
                        nc.tensor.transpose(tp2[:], avs[:, kc * P:(kc + 1) * P],
                                            ident[:])
                        t_ = att.tile([P, P], bf16, tag="AXT2", bufs=4)
                        nc.scalar.copy(t_[:], tp2[:])
                        AXT2.append(t_)
                    for kc in range(KC):
                        nc.tensor.matmul(
                            ops2[:, h * dh:(h + 1) * dh],
                            lhsT=AXT2[kc][:],
                            rhs=eW[2][:, kc * D + h * dh:kc * D + (h + 1) * dh],
                            start=False,
                            stop=(h == H_HEADS - 1 and kc == KC - 1))
                v_c = att.tile([P, D], f32, tag="v_c")
                nc.vector.tensor_tensor(out=v_c[:], in0=ops2[:], in1=q_s[:],
                                        op=ALU.add)
                t3 = att.tile([P, D], f32, tag="lnt1", bufs=1)
                ln_norm(v_c, elng[0][0], elng[0][1], t3)
                t4 = att.tile([P, D], f32, tag="lnt2", bufs=1)
                ffn_block(t3, eW[3], eb3, t4)
                v_f = vnsp.tile([P, D], bf16, tag="v_f")
                ln_norm(t4, elng[1][0], elng[1][1], v_f)
                vf4[G] = v_f

            # ---- 5b. exchange A slices ----
            nc.gpsimd.collective_compute(
                "AllGather", ALU.bypass,
                replica_groups=[list(range(NCORES))],
                ins=[ag2_in[:]], outs=[ag2_out[:]])

            # ---- 7. smix + combT: xT4 += (vns/H)^T @ smix ----
            # gather each graph's 4 A-slices (128 contiguous rows, per-core
            # row base in idxw2) with two 1024-row dma_gathers
            sm2 = []
            for half in (0, 1):
                t = smp.tile([P, 8 * MAX_N], bf16, tag="sm2")
                nc.gpsimd.dma_gather(
                    t[:].rearrange("p (c d) -> p c d", d=MAX_N),
                    ag2_out[:], idxw2_t[:, half * 64:(half + 1) * 64],
                    8 * P, 8 * P, MAX_N, queue_num=half)
                sm2.append(t)
            for G in range(NGRP):
                SMIX4 = smp.tile([P, GW], bf16, tag="smix4")
                for gg in range(4):
                    g = 4 * G + gg
                    mix = psB.tile([P, MAX_N], f32, tag="psB")
                    nc.tensor.matmul(
                        mix[:], lhsT=selmix_t[gg][:],
                        rhs=sm2[g // 8][:, (g % 8) * MAX_N:(g % 8 + 1) * MAX_N],
                        start=True, stop=True)
                    nc.scalar.copy(SMIX4[:, gg * MAX_N:(gg + 1) * MAX_N],
                                   mix[:])
                for kc in range(KC):
                    for cc in range(GW // 512):
                        ps = psB.tile([P, 512], f32, tag="psB")
                        nc.tensor.matmul(
                            ps[:], lhsT=vf4[G][:, kc * P:(kc + 1) * P],
                            rhs=SMIX4[:, cc * 512:(cc + 1) * 512],
                            start=True, stop=True)
                        nc.vector.tensor_tensor(
                            out=xT4[G][kc][:, cc * 512:(cc + 1) * 512],
                            in0=xT4[G][kc][:, cc * 512:(cc + 1) * 512],
                            in1=ps[:], op=ALU.add)

        # ============================ head ============================
        for b in range(NBLK):
            G, rest = b // (4 * NT), b % (4 * NT)
            ps = psT.tile([P, D_OUT], f32, tag="tr")
            for kc in range(KC):
                nc.tensor.matmul(ps[:],
                                 lhsT=xT4[G][kc][:, rest * P:(rest + 1) * P],
                                 rhs=headW_t[:, kc * D_OUT:(kc + 1) * D_OUT],
                                 start=(kc == 0), stop=(kc == KC - 1))
            yt = att.tile([P, D_OUT], f32, tag="yt")
            nc.vector.tensor_tensor(out=yt[:], in0=ps[:], in1=headb_t[:],
                                    op=ALU.add)
            nc.sync.dma_start(y_d[b * P:(b + 1) * P, :], yt[:])

    nc.compile()
    return nc


# ----------------------------------------------------------------------------
# Input maps
# ----------------------------------------------------------------------------

def _make_in_maps(np_inputs, meta, pre):
    proj_ln_g = np_inputs["proj_ln_g"]
    proj_ln_b = np_inputs["proj_ln_b"]
    exch_ln_g = np_inputs["exch_ln_g"]
    exch_ln_b = np_inputs["exch_ln_b"]
    gcn_W = np_inputs["gcn_W"]
    gcn_b = np_inputs["gcn_b"]
    proj_W = np_inputs["proj_W"]
    proj_b = np_inputs["proj_b"]
    exch_W = np_inputs["exch_W"]
    exch_b = np_inputs["exch_b"]
    head_W = np_inputs["head_W"]
    head_b = np_inputs["head_b"]

    bcast = lambda v: np.broadcast_to(np.asarray(v, np.float32), (P, D)).copy()
    pln = np.zeros((L_LAYERS, 2, 2, P, D), np.float32)
    eln = np.zeros((L_LAYERS, 2, 2, P, D), np.float32)
    for l in range(L_LAYERS):
        for i in range(2):
            pln[l, i, 0] = bcast(proj_ln_g[l][i])
            pln[l, i, 1] = bcast(proj_ln_b[l][i])
            eln[l, i, 0] = bcast(exch_ln_g[l][i])
            eln[l, i, 1] = bcast(exch_ln_b[l][i])

    colT = lambda v: np.asarray(v, np.float32).reshape(L_LAYERS, KC, P) \
        .transpose(0, 2, 1).copy()
    pb = np.asarray(proj_b, np.float32)
    eb = np.asarray(exch_b, np.float32)
    b16 = lambda v: np.ascontiguousarray(v).astype(ml_dtypes.bfloat16)
    shared = dict(
        qbd=b16(pre["qbd"]), qcat4=pre["qcat4"],
        sabmask=pre["sabmask"], selmix=b16(pre["selmix"]),
        gcnW=b16(np.asarray(gcn_W, np.float32)),
        gcnb=b16(np.asarray(gcn_b, np.float32).reshape(L_LAYERS, 1, D)),
        pW1=b16(np.asarray(proj_W, np.float32)[:, 1]),
        pb1T=colT(pb[:, 1]),
        pW2=b16(np.asarray(proj_W, np.float32)[:, 2]),
        pb2=b16(pb[:, 2].reshape(L_LAYERS, 1, D)),
        pW3=b16(np.asarray(proj_W, np.float32)[:, 3]),
        pb3=b16(pb[:, 3].reshape(L_LAYERS, 1, D)),
        pln=pln,
        eW=b16(np.asarray(exch_W, np.float32)),
        eb0=b16(eb[:, 0].reshape(L_LAYERS, 1, D)),
        eb0T=colT(eb[:, 0]),
        eb1T=colT(eb[:, 1]),
        eb2=b16(eb[:, 2].reshape(L_LAYERS, 1, D)),
        eb3=b16(eb[:, 3].reshape(L_LAYERS, 1, D)),
        eln=eln,
        headW=b16(np.asarray(head_W, np.float32)),
        headb=np.broadcast_to(np.asarray(head_b, np.float32),
                              (P, D_OUT)).copy(),
    )
    in_maps = []
    for c in range(NCORES):
        m = dict(shared)
        m["xT0"] = pre["xT"][c].astype(ml_dtypes.bfloat16)
        m["idxw"] = pre["idxw"][c]
        m["idxw2"] = pre["idxw2"][c]
        m["eldst"] = pre["eldst"][c]
        m["ew"] = pre["ew"][c]
        m["negmask"] = pre["negmask"][c].astype(ml_dtypes.bfloat16)
        in_maps.append(m)
    return in_maps


# ----------------------------------------------------------------------------
# NTFF profiling hook shim (device-side exec time under axon)
# ----------------------------------------------------------------------------

def _install_profile_hook():
    try:
        import antenv
        try:
            from antenv.axon_hooks import get_axon_ntff_profile_hook
            if get_axon_ntff_profile_hook() is not None:
                return True
        except ImportError:
            _store = {"h": None}
            mod = types.ModuleType("antenv.axon_hooks")
            mod.set_axon_ntff_profile_hook = lambda h: _store.update(h=h)
            mod.get_axon_ntff_profile_hook = lambda: _store["h"]
            sys.modules["antenv.axon_hooks"] = mod
            antenv.axon_hooks = mod
        if "/root/.axon_site" not in sys.path:
            sys.path.append("/root/.axon_site")
        from trn_agent_boot.trn_boot import _ntff_profile_via_ctypes
        hook = _ntff_profile_via_ctypes("/opt/axon/libaxon_pjrt.so")
        if hook is None:
            return False
        from antenv.axon_hooks import set_axon_ntff_profile_hook
        set_axon_ntff_profile_hook(hook)
        import concourse.bass_utils as BU
        BU.upload_artifacts = lambda tmpdir: f"local:{tmpdir}"
        return True
    except Exception:
        return False


# ----------------------------------------------------------------------------
# Entry point
# ----------------------------------------------------------------------------

def kernel(x, gcn_W, gcn_b, seeds, proj_W, proj_b, proj_ln_g, proj_ln_b,
           exch_W, exch_b, exch_ln_g, exch_ln_b, head_W, head_b,
           edge_index, batch_ids):
    global LAST_EXEC_TIME_NS
    meta, pre = _preprocess(x, edge_index, batch_ids, seeds, proj_W, proj_b)

    if "nc" not in _CACHE:
        _CACHE["nc"] = _build(meta)
    nc = _CACHE["nc"]

    np_inputs = dict(
        gcn_W=gcn_W, gcn_b=gcn_b, proj_W=proj_W, proj_b=proj_b,
        proj_ln_g=proj_ln_g, proj_ln_b=proj_ln_b, exch_W=exch_W,
        exch_b=exch_b, exch_ln_g=exch_ln_g, exch_ln_b=exch_ln_b,
        head_W=head_W, head_b=head_b)
    in_maps = _make_in_maps(np_inputs, meta, pre)

    mode = os.environ.get("ANT_BENCH", "trace")
    results = None
    if mode == "trace" and _install_profile_hook():
        import tempfile
        tmpdir = tempfile.mkdtemp(prefix="ktrace_")
        try:
            res = run_bass_kernel_spmd(nc, in_maps, list(range(NCORES)),
                                       trace=True, tmpdir=tmpdir)
            results = res.results
            LAST_EXEC_TIME_NS = res.exec_time_ns
        except Exception:
            results = None
    if results is None:
        if mode == "wall":
            results, tmin = _run_pjrt_timed(
                nc, in_maps, NCORES,
                iters=int(os.environ.get("ANT_BENCH_ITERS", "3")))
            LAST_EXEC_TIME_NS = int(tmin * 1e9) if tmin else None
        else:
            res = run_bass_kernel_spmd(nc, in_maps, list(range(NCORES)))
            results = res.results
            LAST_EXEC_TIME_NS = res.exec_time_ns

    gcore, dslot = meta["gcore"], meta["dslot"]
    y = np.zeros((N_NODES, D_OUT), np.float32)
    for c in range(NCORES):
        idx = np.where(gcore == c)[0]
        y[idx] = results[c]["y"][dslot[idx]]
    return y


# ----------------------------------------------------------------------------
# Timed PJRT runner (jit once, time warm iterations) — wall-clock fallback
# ----------------------------------------------------------------------------

def _run_pjrt_timed(nc, in_maps, n_cores, iters=2):
    import time as _time
    import jax
    from jax.experimental.shard_map import shard_map
    from jax.sharding import Mesh, NamedSharding, PartitionSpec
    from concourse import bass2jax as B
    from concourse import mybir as mb

    B.install_neuronx_cc_hook()
    partition_name = (nc.partition_id_tensor.name
                      if nc.partition_id_tensor else None)
    in_names, out_names, out_avals, zero_shapes = [], [], [], []
    for alloc in nc.m.functions[0].allocations:
        if not isinstance(alloc, mb.MemoryLocationSet):
            continue
        name = alloc.memorylocations[0].name
        if alloc.kind == "ExternalInput":
            if name != partition_name:
                in_names.append(name)
        elif alloc.kind == "ExternalOutput":
            shape = tuple(alloc.tensor_shape)
            dtype = mb.dt.np(alloc.dtype)
            out_names.append(name)
            out_avals.append(jax.core.ShapedArray(shape, dtype))
            zero_shapes.append((shape, dtype))
    n_params = len(in_names)
    n_outs = len(out_names)
    all_in = list(in_names) + list(out_names)
    if partition_name is not None:
        all_in.append(partition_name)
    donate = tuple(range(n_params, n_params + n_outs))

    def _body(*args):
        operands = list(args)
        if partition_name is not None:
            operands.append(B.partition_id_tensor())
        return tuple(B._bass_exec_p.bind(
            *operands, out_avals=tuple(out_avals), in_names=tuple(all_in),
            out_names=tuple(out_names), lowering_input_output_aliases=(),
            sim_require_finite=True, sim_require_nnan=True, nc=nc))

    devices = jax.devices()[:n_cores]
    mesh = Mesh(np.asarray(devices), ("core",))
    sh = NamedSharding(mesh, PartitionSpec("core"))
    in_specs = (PartitionSpec("core"),) * (n_params + n_outs)
    out_specs = (PartitionSpec("core"),) * n_outs
    sharded = jax.jit(
        shard_map(_body, mesh=mesh, in_specs=in_specs, out_specs=out_specs,
                  check_rep=False),
        donate_argnums=donate, keep_unused=True)

    dev_in = [
        jax.device_put(
            np.concatenate([np.asarray(in_maps[c][n]) for c in range(n_cores)],
                           axis=0), sh)
        for n in in_names
    ]

    def zeros():
        return [jax.device_put(
            np.zeros((n_cores * s[0], *s[1:]), d), sh)
            for s, d in zero_shapes]

    outs = sharded(*dev_in, *zeros())
    outs = [np.asarray(o) for o in outs]
    times = []
    for _ in range(iters):
        z = zeros()
        jax.block_until_ready(z)
        t0 = _time.perf_counter()
        o2 = sharded(*dev_in, *z)
        jax.block_until_ready(o2)
        times.append(_time.perf_counter() - t0)
        del o2
    results = [
        {name: outs[i].reshape(n_cores, *zero_shapes[i][0])[c]
         for i, name in enumerate(out_names)}
        for c in range(n_cores)
    ]
    return results, (min(times) if times else None)
